# revision 1
# baseline (speedup 1.0000x reference)
"""Trainium2 Bass kernel for nn_Block_27384711479862 (ConvNeXt-ish metaformer block).

Per-core computation (data parallel over batch B=8 -> 8 cores):
  x: [C=384, N=2304]  (N = 48*48 spatial)
  attention branch (bn1 + qkv + softmax + proj, all folded into weights):
      q = qwT.T @ x + qb ; k = kwT.T @ x + kb            (bf16 matmuls)
      vT[n, c] = x.T @ vwT                   (bias folded out via softmax-sum=1)
      s[n, m] = q[:,n].k[:,m];  a = exp(s/sqrt(C))  (no max-sub; logits bounded)
      aT[m, n] = a[n, m] / l[n]      (1/l folded into PE transpose via diag(r))
      attn[c, n] = sum_m vT[m, c] aT[m, n]
      x1 = alpha1*x + pwT.T @ attn   (beta1 deferred to the final residual add)
  mlp branch (bn2 + fc1 + dwconv3x3 + gelu + fc2, folded):
      h = fc1wT.T @ x1 + fc1b  -> stored zero-padded [128, 50, 50] per chunk
      dw = sum_{9 taps} diag(w_tap) @ shifted(h)   (PSUM accumulate on PE for
           10 chunks; chunks 10-11 as chained mul-add on the Vector engine)
      g = gelu(dw + dwb)
      out = (fc2wT.T @ g + (beta1 + fc2b)) + x1
"""
import numpy as np
import ml_dtypes

C = 384
HID = 1536
H = W = 48
N = H * W              # 2304
NC_ = 3                # C chunks of 128
NH = 12                # HID chunks of 128
NRB = 18               # row blocks of 128 queries
EPS = 1e-5
BF16 = ml_dtypes.bfloat16

# n-tiles for the attention side (PSUM bank = 512 f32)
NT5 = [(i * 512, min(512, N - i * 512)) for i in range((N + 511) // 512)]
# n-tiles for the MLP side: 6 tiles of 8 spatial rows (384 cols)
NT6 = [(i * 384, 384) for i in range(6)]
ROWS_PER_TILE = 8
PAD = 50               # padded spatial row stride

_PROG = None           # cached compiled program


def _build_program(iters=1):
    import concourse.bacc as bacc
    import concourse.bass as bass
    import concourse.mybir as mybir
    import concourse.tile as tile
    from contextlib import ExitStack

    dt = mybir.dt
    AF = mybir.ActivationFunctionType
    ALU = mybir.AluOpType
    f32, f32r, bf16 = dt.float32, dt.float32r, dt.bfloat16

    nc = bacc.Bacc("TRN2", target_bir_lowering=False, debug=False,
                   enable_asserts=False)

    def din(name, shape, d=f32):
        return nc.dram_tensor(name, list(shape), d, kind="ExternalInput").ap()

    x_d = din("x", (C, N))
    qwT_d = din("qwT", (C, C), bf16)
    kwT_d = din("kwT", (C, C), bf16)
    vwT_d = din("vwT", (C, C), bf16)
    pwT_d = din("pwT", (C, C), bf16)
    f1wT_d = din("fc1wT", (C, HID), bf16)
    f2wT_d = din("fc2wT", (HID, C), bf16)
    dww_d = din("dww", (128, NH * 9))
    qb_d = din("qb", (128, NC_))
    kb_d = din("kb", (128, NC_))
    f1b_d = din("fc1b", (128, NH))
    dwb_d = din("dwb", (128, NH))
    al1_d = din("alpha1", (128, NC_))
    resb_d = din("resb", (128, NC_))
    iden_d = din("iden", (128, 128), bf16)
    out_d = nc.dram_tensor("out", [C, N], f32, kind="ExternalOutput").ap()
    chain = [x_d]
    for i in range(1, iters):
        chain.append(nc.dram_tensor(f"mid{i}", [C, N], f32).ap())
    chain.append(out_d)

    inv_sqrt_c = float(1.0 / np.sqrt(np.float32(C)))

    with tile.TileContext(nc) as tc:
      for it in range(iters):
        x_d, out_d = chain[it], chain[it + 1]
        with ExitStack() as top:
          # ---- persistent pools -------------------------------------------
          consts = top.enter_context(tc.tile_pool(name="consts", bufs=1))
          pmm = top.enter_context(tc.tile_pool(name="pmm", bufs=2, space="PSUM"))
          x1p = top.enter_context(tc.tile_pool(name="x1p", bufs=1))

          def load_const(ap, shape, d=f32, tag=None):
              t = consts.tile(list(shape), d, tag=tag, name=tag)
              nc.sync.dma_start(t[:], ap)
              return t


          x1_t = [x1p.tile([128, N], f32, tag=f"x1_{c}", name=f"x1_{c}") for c in range(NC_)]
          x1b_t = [x1p.tile([128, N], bf16, tag=f"x1b_{c}", name=f"x1b_{c}")
                   for c in range(NC_)]

          with ExitStack() as attn_scope:
              wq = attn_scope.enter_context(tc.tile_pool(name="wq", bufs=1))
              xp = attn_scope.enter_context(tc.tile_pool(name="xp", bufs=1))
              qkp = attn_scope.enter_context(tc.tile_pool(name="qkp", bufs=1))
              vTp = attn_scope.enter_context(tc.tile_pool(name="vTp", bufs=1))

              qwT_s = [wq.tile([128, C], bf16, tag=f"qw{k}", name=f"qw{k}") for k in range(NC_)]
              kwT_s = [wq.tile([128, C], bf16, tag=f"kw{k}", name=f"kw{k}") for k in range(NC_)]
              vwT_s = [wq.tile([128, C], bf16, tag=f"vw{k}", name=f"vw{k}") for k in range(NC_)]
              pwT_s = [wq.tile([128, C], bf16, tag=f"pw{k}", name=f"pw{k}") for k in range(NC_)]
              for k in range(NC_):
                  sl = slice(k * 128, (k + 1) * 128)
                  nc.sync.dma_start(qwT_s[k][:], qwT_d[sl, :])
                  nc.sync.dma_start(kwT_s[k][:], kwT_d[sl, :])

              x_t = [xp.tile([128, N], f32, tag=f"x_{c}", name=f"x_{c}") for c in range(NC_)]
              for ti, (n0, nn) in enumerate(NT5):
                  for c in range(NC_):
                      nc.sync.dma_start(x_t[c][:, n0:n0 + nn],
                                        x_d[c * 128:(c + 1) * 128, n0:n0 + nn])
                  if ti == 0:
                      # rest of the attention weights + consts after x tile 0
                      for k in range(NC_):
                          sl = slice(k * 128, (k + 1) * 128)
                          nc.sync.dma_start(vwT_s[k][:], vwT_d[sl, :])
                          nc.sync.dma_start(pwT_s[k][:], pwT_d[sl, :])
                      qb_s = load_const(qb_d, (128, NC_), tag="qb")
                      kb_s = load_const(kb_d, (128, NC_), tag="kb")
                      al1_s = load_const(al1_d, (128, NC_), tag="al1")
                      iden_s = load_const(iden_d, (128, 128), bf16, tag="iden")
                      resb_s = load_const(resb_d, (128, NC_), tag="resb")
                      f1b_s = load_const(f1b_d, (128, NH), tag="f1b")
                      dwb_s = load_const(dwb_d, (128, NH), tag="dwb")
                      dww_s = load_const(dww_d, (128, NH * 9), tag="dww")

              q_t = [qkp.tile([128, N], bf16, tag=f"q_{c}", name=f"q_{c}") for c in range(NC_)]
              k_t = [qkp.tile([128, N], bf16, tag=f"k_{c}", name=f"k_{c}") for c in range(NC_)]
              xbf_t = [qkp.tile([128, N], bf16, tag=f"xbf_{c}", name=f"xbf_{c}")
                       for c in range(NC_)]
              for (n0, nn) in NT5:
                  for c in range(NC_):
                      nc.scalar.copy(xbf_t[c][:, n0:n0 + nn],
                                     x_t[c][:, n0:n0 + nn])

              # ---- q, k = w.T @ x + b  (f32r) -----------------------------
              for mc in range(NC_):
                  msl = slice(mc * 128, (mc + 1) * 128)
                  for (n0, nn) in NT5:
                      for which, wt, bt, dst in (
                          (0, qwT_s, qb_s, q_t), (1, kwT_s, kb_s, k_t)):
                          ps = pmm.tile([128, 512], f32, tag="mm", name="mm")
                          for kc in range(NC_):
                              nc.tensor.matmul(
                                  ps[:, :nn],
                                  wt[kc][:, msl],
                                  xbf_t[kc][:, n0:n0 + nn],
                                  start=(kc == 0), stop=(kc == NC_ - 1))
                          nc.vector.tensor_scalar_add(
                              dst[mc][:, n0:n0 + nn], ps[:, :nn],
                              bt[:, mc:mc + 1])

              # ---- vT[n, c] = x.T @ vwT  (f32r, no bias) ------------------
              vT_t = [vTp.tile([128, C], bf16, tag=f"vT_{b}", name=f"vT_{b}") for b in range(NRB)]
              for nb in range(NRB):
                  ps = pmm.tile([128, 512], f32, tag="mm", name="mm")
                  for kc in range(NC_):
                      nc.tensor.matmul(
                          ps[:, :C],
                          xbf_t[kc][:, nb * 128:(nb + 1) * 128],
                          vwT_s[kc][:],
                          start=(kc == 0), stop=(kc == NC_ - 1))
                  nc.vector.tensor_copy(vT_t[nb][:], ps[:, :C])

              # ---- attention groups (4 row-blocks = 512 queries each) -----
              with ExitStack() as grp_scope:
                  ap_ = grp_scope.enter_context(tc.tile_pool(name="ap", bufs=5))
                  aTp = grp_scope.enter_context(tc.tile_pool(name="aTp", bufs=1))
                  stp = grp_scope.enter_context(tc.tile_pool(name="stp", bufs=6))
                  anp = grp_scope.enter_context(tc.tile_pool(name="anp", bufs=4))
                  psc = grp_scope.enter_context(
                      tc.tile_pool(name="psc", bufs=2, space="PSUM"))
                  ptr = grp_scope.enter_context(
                      tc.tile_pool(name="ptr", bufs=2, space="PSUM"))
                  pat = grp_scope.enter_context(
                      tc.tile_pool(name="pat", bufs=2, space="PSUM"))

                  groups = [list(range(g, min(g + 4, NRB)))
                            for g in range(0, NRB, 4)]
                  for grp in groups:
                      gw = 128 * len(grp)           # group width (queries)
                      g0 = grp[0] * 128
                      aT_t = [aTp.tile([128, gw], bf16, tag=f"aT_{t}", name=f"aT_{t}")
                              for t in range(NRB)]
                      diag_t = []
                      for gi, rb in enumerate(grp):
                          rsl = slice(rb * 128, (rb + 1) * 128)
                          a_t = ap_.tile([128, N], bf16, tag="a", name="a")
                          lsum = stp.tile([128, len(NT5)], f32, tag="ls", name="ls")
                          for ti, (n0, nn) in enumerate(NT5):
                              ps = psc.tile([128, 512], f32, tag="sc", name="sc")
                              for kc in range(NC_):
                                  nc.tensor.matmul(
                                      ps[:, :nn], q_t[kc][:, rsl],
                                      k_t[kc][:, n0:n0 + nn],
                                      start=(kc == 0), stop=(kc == NC_ - 1))
                              nc.scalar.activation(
                                  a_t[:, n0:n0 + nn], ps[:, :nn], AF.Exp,
                                  scale=inv_sqrt_c,
                                  accum_out=lsum[:, ti:ti + 1])
                          lt = stp.tile([128, 1], f32, tag="l", name="l")
                          rt = stp.tile([128, 1], f32, tag="r", name="r")
                          dg = stp.tile([128, 128], bf16, tag="dg", name="dg")
                          nc.vector.reduce_sum(lt[:], lsum[:], axis=mybir.AxisListType.X)
                          nc.vector.reciprocal(rt[:], lt[:])
                          nc.vector.tensor_scalar_mul(dg[:], iden_s[:], rt[:])
                          diag_t.append((a_t, dg))

                      # transposes: pairs of row blocks -> one PSUM tile
                      for p0 in range(0, len(grp), 2):
                          pw = 128 * min(2, len(grp) - p0)
                          for t in range(NRB):
                              tp = ptr.tile([128, 256], f32, tag="tr", name="tr")
                              for gi in range(p0, min(p0 + 2, len(grp))):
                                  a_t, dg = diag_t[gi]
                                  nc.tensor.matmul(
                                      tp[:, (gi - p0) * 128:(gi - p0 + 1) * 128],
                                      a_t[:, t * 128:(t + 1) * 128], dg[:],
                                      start=True, stop=True)
                              nc.vector.tensor_copy(
                                  aT_t[t][:, p0 * 128:p0 * 128 + pw],
                                  tp[:, :pw])

                      # attn[c, n] = sum_m vT[m, c-chunk] @ aT[m, n]
                      attn_t = []
                      for mc in range(NC_):
                          pa = pat.tile([128, 512], f32, tag="at", name="at")
                          for t in range(NRB):
                              nc.tensor.matmul(
                                  pa[:, :gw],
                                  vT_t[t][:, mc * 128:(mc + 1) * 128],
                                  aT_t[t][:], start=(t == 0), stop=(t == NRB - 1))
                          ab = anp.tile([128, 512], bf16, tag="an", name="an")
                          nc.vector.tensor_copy(ab[:, :gw], pa[:, :gw])
                          attn_t.append(ab)

                      # proj + residual: x1 = alpha1*x + beta1 + pwT.T@attn
                      for mc in range(NC_):
                          msl = slice(mc * 128, (mc + 1) * 128)
                          ps = pmm.tile([128, 512], f32, tag="mm", name="mm")
                          for kc in range(NC_):
                              nc.tensor.matmul(
                                  ps[:, :gw], pwT_s[kc][:, msl],
                                  attn_t[kc][:, :gw], start=(kc == 0),
                                  stop=(kc == NC_ - 1))
                          nc.vector.scalar_tensor_tensor(
                              x1_t[mc][:, g0:g0 + gw], x_t[mc][:, g0:g0 + gw],
                              al1_s[:, mc:mc + 1], ps[:, :gw],
                              op0=ALU.mult, op1=ALU.add)
                          nc.vector.tensor_copy(x1b_t[mc][:, g0:g0 + gw],
                                                x1_t[mc][:, g0:g0 + gw])

          # ---- MLP ---------------------------------------------------------
          with ExitStack() as mlp_scope:
              wm = mlp_scope.enter_context(tc.tile_pool(name="wm", bufs=1))
              hp = mlp_scope.enter_context(tc.tile_pool(name="hp", bufs=1))
              gp = mlp_scope.enter_context(tc.tile_pool(name="gp", bufs=3))
              dgp = mlp_scope.enter_context(tc.tile_pool(name="dgp", bufs=1))
              outp = mlp_scope.enter_context(tc.tile_pool(name="outp", bufs=4))
              accp = mlp_scope.enter_context(tc.tile_pool(name="accp", bufs=3))
              pdw = mlp_scope.enter_context(
                  tc.tile_pool(name="pdw", bufs=2, space="PSUM"))

              f1wT_s = [wm.tile([128, HID], bf16, tag=f"f1w{k}", name=f"f1w{k}") for k in range(NC_)]
              for k in range(NC_):
                  nc.sync.dma_start(f1wT_s[k][:], f1wT_d[k * 128:(k + 1) * 128, :])
              f2wT_s = [wm.tile([128, C], bf16, tag=f"f2w{k}", name=f"f2w{k}") for k in range(NH)]
              for k in range(NH):
                  nc.sync.dma_start(f2wT_s[k][:], f2wT_d[k * 128:(k + 1) * 128, :])

              # padded h: [128, 50, 50] per HID chunk, borders zeroed
              h_t = [hp.tile([128, PAD * PAD], bf16, tag=f"h_{c}", name=f"h_{c}") for c in range(NH)]
              for c in range(NH):
                  hv = h_t[c][:].rearrange("p (y x) -> p y x", y=PAD)
                  nc.gpsimd.memset(hv[:, 0, :], 0.0)
                  nc.gpsimd.memset(hv[:, PAD - 1, :], 0.0)
                  nc.gpsimd.memset(hv[:, :, 0], 0.0)
                  nc.gpsimd.memset(hv[:, :, PAD - 1], 0.0)

              # fc1 -> h (padded, bf16, bias via ACT)
              for ti, (n0, nn) in enumerate(NT6):
                  y0 = ti * ROWS_PER_TILE
                  for hc in range(NH):
                      ps = pmm.tile([128, 512], f32, tag="mm", name="mm")
                      for kc in range(NC_):
                          nc.tensor.matmul(
                              ps[:, :nn],
                              f1wT_s[kc][:, hc * 128:(hc + 1) * 128],
                              x1b_t[kc][:, n0:n0 + nn],
                              start=(kc == 0), stop=(kc == NC_ - 1))
                      dst = h_t[hc][:].rearrange(
                          "p (y x) -> p y x", y=PAD)[
                          :, y0 + 1:y0 + 1 + ROWS_PER_TILE, 1:1 + W]
                      psv = ps[:, :nn].rearrange("p (y x) -> p y x",
                                                 y=ROWS_PER_TILE)
                      if hc % 2 == 0:
                          nc.scalar.activation(dst, psv, AF.Identity,
                                               bias=f1b_s[:, hc:hc + 1])
                      else:
                          nc.vector.tensor_scalar_add(dst, psv,
                                                      f1b_s[:, hc:hc + 1])

              # dwconv diag weights: dg[c][tap] = iden * w  (bf16)
              PE_DW = 10         # base split; tile 0 keeps 10 chunks on PE
              dwdiag = [[None] * 9 for _ in range(NH)]
              for hc in range(10):
                  for tap in range(9):
                      d = dgp.tile([128, 128], bf16, tag=f"dwd_{hc}_{tap}", name=f"dwd_{hc}_{tap}")
                      nc.vector.tensor_scalar_mul(
                          d[:], iden_s[:], dww_s[:, hc * 9 + tap:hc * 9 + tap + 1])
                      dwdiag[hc][tap] = d

              # dwconv (9 shifted diag matmuls) + gelu -> g ; fc2 + residual
              for ti, (n0, nn) in enumerate(NT6):
                  y0 = ti * ROWS_PER_TILE
                  g_t = []
                  for hc in range(NH):
                      hv = h_t[hc][:].rearrange("p (y x) -> p y x", y=PAD)
                      g = gp.tile([128, 384], bf16, tag=f"g_{hc}", name=f"g_{hc}")
                      pe_dw = 10 if ti == 0 else (8 if ti >= 4 else PE_DW)
                      if hc < pe_dw:
                          ps = pdw.tile([128, 512], f32, tag="dw", name="dw")
                          for tap in range(9):
                              dy, dx = divmod(tap, 3)
                              rhs = hv[:, y0 + dy:y0 + dy + ROWS_PER_TILE, dx:dx + W]
                              nc.tensor.matmul(
                                  ps[:, :nn].rearrange("p (y x) -> p y x",
                                                       y=ROWS_PER_TILE),
                                  dwdiag[hc][tap][:], rhs,
                                  start=(tap == 0), stop=(tap == 8))
                          nc.scalar.activation(g[:], ps[:, :nn], AF.Gelu,
                                               bias=dwb_s[:, hc:hc + 1])
                      else:
                          acc = accp.tile([128, 384], f32, tag=f"acc_{hc}",
                                          name=f"acc_{hc}")
                          av = acc[:].rearrange("p (y x) -> p y x",
                                                y=ROWS_PER_TILE)
                          for tap in range(9):
                              dy, dx = divmod(tap, 3)
                              rhs = hv[:, y0 + dy:y0 + dy + ROWS_PER_TILE,
                                       dx:dx + W]
                              wcol = dww_s[:, hc * 9 + tap:hc * 9 + tap + 1]
                              if tap == 0:
                                  nc.vector.tensor_scalar_mul(av, rhs, wcol)
                              else:
                                  nc.vector.scalar_tensor_tensor(
                                      av, rhs, wcol, av,
                                      op0=ALU.mult, op1=ALU.add)
                          nc.scalar.activation(g[:], acc[:], AF.Gelu,
                                               bias=dwb_s[:, hc:hc + 1])
                      g_t.append(g)
                  for mc in range(NC_):
                      msl = slice(mc * 128, (mc + 1) * 128)
                      ps = pmm.tile([128, 512], f32, tag="mm", name="mm")
                      for hc in range(NH):
                          nc.tensor.matmul(ps[:, :nn], f2wT_s[hc][:, msl],
                                           g_t[hc][:], start=(hc == 0),
                                           stop=(hc == NH - 1))
                      ot = outp.tile([128, 384], f32, tag="ot", name="ot")
                      nc.vector.scalar_tensor_tensor(
                          ot[:, :nn], ps[:, :nn], resb_s[:, mc:mc + 1],
                          x1_t[mc][:, n0:n0 + nn], op0=ALU.add, op1=ALU.add)
                      nc.sync.dma_start(out_d[msl, n0:n0 + nn], ot[:, :nn])

    nc.compile()
    return nc


def _fold_inputs(inputs):
    """Host-side weight folding. Returns (shared weight map, per-core x list)."""
    f = np.float32
    g = {k: np.asarray(v, f) for k, v in inputs.items()}
    s1 = g['bn1_g'] / np.sqrt(g['bn1_v'] + EPS)
    t1 = g['bn1_b'] - g['bn1_m'] * s1
    qw = g['q_w'] * s1[None, :]; qb = g['q_w'] @ t1 + g['q_b']
    kw = g['k_w'] * s1[None, :]; kb = g['k_w'] @ t1 + g['k_b']
    vw = g['v_w'] * s1[None, :]
    vb_eff = g['v_w'] @ t1 + g['v_b']
    ls1, ls2 = g['ls1'], g['ls2']
    pw = ls1[:, None] * g['po_w']
    alpha1 = 1.0 + ls1 * s1
    beta1 = ls1 * (g['po_b'] + t1) + pw @ vb_eff
    s2 = g['bn2_g'] / np.sqrt(g['bn2_v'] + EPS)
    t2 = g['bn2_b'] - g['bn2_m'] * s2
    f1w = g['fc1_w'] * s2[None, :]
    f1b = g['fc1_w'] @ t2 + g['fc1_b']
    f2w = ls2[:, None] * g['fc2_w']
    f2b = ls2 * g['fc2_b']
    f1b = f1b + f1w @ beta1          # x1 on device excludes beta1
    resb = beta1 + f2b
    dww = g['dw_w'].reshape(HID, 9)

    w = {
        'qwT': np.ascontiguousarray(qw.T).astype(BF16),
        'kwT': np.ascontiguousarray(kw.T).astype(BF16),
        'vwT': np.ascontiguousarray(vw.T).astype(BF16),
        'pwT': np.ascontiguousarray(pw.T).astype(BF16),
        'fc1wT': np.ascontiguousarray(f1w.T).astype(BF16),
        'fc2wT': np.ascontiguousarray(f2w.T).astype(BF16),
        'dww': np.ascontiguousarray(dww.reshape(NH, 128, 9).transpose(1, 0, 2)
                                    .reshape(128, NH * 9)),
        'qb': np.ascontiguousarray(qb.reshape(NC_, 128).T),
        'kb': np.ascontiguousarray(kb.reshape(NC_, 128).T),
        'fc1b': np.ascontiguousarray(f1b.reshape(NH, 128).T),
        'dwb': np.ascontiguousarray(g['dw_b'].reshape(NH, 128).T),
        'alpha1': np.ascontiguousarray(alpha1.reshape(NC_, 128).T),
        'resb': np.ascontiguousarray(resb.reshape(NC_, 128).T),
        'iden': np.eye(128, dtype=BF16),
    }
    xs = [np.ascontiguousarray(g['x'][b].reshape(C, N)) for b in range(g['x'].shape[0])]
    return w, xs


def get_program():
    global _PROG
    if _PROG is None:
        _PROG = _build_program()
    return _PROG


def kernel(**inputs):
    from concourse.bass_utils import run_bass_kernel_spmd
    nc = get_program()
    w, xs = _fold_inputs(inputs)
    B = len(xs)
    in_maps = [{**w, 'x': xs[b]} for b in range(B)]
    res = run_bass_kernel_spmd(nc, in_maps, list(range(B)))
    out = np.stack([res.results[b]['out'].reshape(C, H, W) for b in range(B)])
    return out.astype(inputs['x'].dtype if hasattr(inputs['x'], 'dtype') else np.float32)



# revision 4
# speedup vs baseline: 1.5833x; 1.5833x over previous
"""Trainium2 Bass kernel for nn_Block_27384711479862 (metaformer block).

fp8e4 DoubleRow rewrite. Per-core computation (data parallel, B=8 -> 8 cores),
x: [C=384, N=2304]:

  attention (bn1/qkv/proj folded on host):
    zhat = GG*(A @ x + u 1^T)          A = kw^T qw,  u = kw^T qb   (fp8, DR)
    E[m,n] = exp(sum_c x[c,m] zhat[c,n] / (GG sqrt(C)) - S)        (Act)
    l = 128 * sum_m E                  (ones-stationary DR matmul)
    r = 1/l                            (= 1/(GV l) with GV=128)
    v'(c,m) = GV * (ls1*po_w@vw) x     (DR, transposed layout)
    x1 = alpha1*x + (v' E) * r         (2 DVE ops; beta1 deferred)
  mlp (bn2 folded):
    h = fp8(fc1 x1 / G1 + f1b)         (DR; padded [50,50] layout, 2 copies)
    dw = 3x3 depthwise via 5 DR tap-pair diag matmuls (pairs span the
         duplicated h copy so moving-AP windows never overlap)
    g = fp8(gelu(dw + dwb))
    out = (fc2 g + resb)/G2 + x1       (DR + K=1 bf16 bias row)
"""
import numpy as np
import ml_dtypes

C = 384
HID = 1536
H = W = 48
N = H * W              # 2304
PAD = 50
EPS = 1e-5
BF16 = ml_dtypes.bfloat16
F8 = ml_dtypes.float8_e4m3

GG = 8.0      # scores/z scale
GV = 128.0    # Wpv scale (must equal the ones-stationary value)
G1 = 16.0     # fc1 scale
G2 = 256.0    # fc2 scale
SQC = float(np.sqrt(np.float32(C)))

NW5 = [(i * 512, min(512, N - i * 512)) for i in range((N + 511) // 512)]
NT6 = [(i * 384, 384) for i in range(6)]
COPY1 = 2560           # second h copy offset inside the [128, 5120] tile
TAPS = [(t // 3, t % 3) for t in range(9)]

_PROG = None


def _build_program(iters=1):
    import concourse.bacc as bacc
    import concourse.mybir as mybir
    import concourse.tile as tile
    import bass_rust
    from contextlib import ExitStack

    dt = mybir.dt
    AF = mybir.ActivationFunctionType
    ALU = mybir.AluOpType
    PM = mybir.MatmulPerfMode
    f32, bf16, fp8 = dt.float32, dt.bfloat16, dt.float8e4

    nc = bacc.Bacc("TRN2", target_bir_lowering=False, debug=False,
                   enable_asserts=False)

    def din(name, shape, d=f32):
        return nc.dram_tensor(name, list(shape), d, kind="ExternalInput").ap()

    x_d = din("x", (C, N))
    xq_d = din("xq", (128, 4 * N), fp8)
    zeros_d = din("zeros8", (128, N), fp8)
    Gt_d = din("Gt", (128, 4 * 384), fp8)
    wpvT_d = din("wpvT", (128, 4 * 384), fp8)
    ones128_d = din("ones128", (128, 256), fp8)
    f1wT_d = din("f1wT", (128, 4 * HID), fp8)
    f2wT_d = din("f2wT", (128, 12 * 384), fp8)
    dwd_d = din("dwd", (128, 15360), fp8)
    ub_d = din("ub", (128, 3))
    sneg_d = din("sneg", (128, 1))
    al1_d = din("al1", (128, 3))
    f1b_d = din("f1b", (128, 12))
    dwb_d = din("dwb", (128, 12))
    resb_d = din("resbr", (1, 384), bf16)
    onesb_d = din("onesb", (1, 384), bf16)
    out_d = nc.dram_tensor("out", [C, N], f32, kind="ExternalOutput").ap()
    chain = [x_d]
    for i in range(1, iters):
        chain.append(nc.dram_tensor(f"mid{i}", [C, N], f32).ap())
    chain.append(out_d)

    exp_scale = float(1.0 / (GG * SQC))

    with tile.TileContext(nc) as tc:
      for it in range(iters):
        x_d, out_d = chain[it], chain[it + 1]
        with ExitStack() as top:
          wp = top.enter_context(tc.tile_pool(name="wp", bufs=1))

          def load(name, shape, d, ap):
              t = wp.tile(list(shape), d, tag=name, name=name)
              nc.sync.dma_start(t[:], ap)
              return t

          Gt_s = load("Gt", (128, 4 * 384), fp8, Gt_d)
          wpvT_s = load("wpvT", (128, 4 * 384), fp8, wpvT_d)
          ones_s = load("ones128", (128, 256), fp8, ones128_d)
          ub_s = load("ub", (128, 3), f32, ub_d)
          sneg_s = load("sneg", (128, 1), f32, sneg_d)
          al1_s = load("al1", (128, 3), f32, al1_d)
          f1wT_s = load("f1wT", (128, 4 * HID), fp8, f1wT_d)
          f2wT_s = load("f2wT", (128, 12 * 384), fp8, f2wT_d)
          dwd_s = load("dwd", (128, 15360), fp8, dwd_d)
          f1b_s = load("f1b", (128, 12), f32, f1b_d)
          dwb_s = load("dwb", (128, 12), f32, dwb_d)
          resb_s = load("resbr", (1, 384), bf16, resb_d)
          onesb_s = load("onesb", (1, 384), bf16, onesb_d)

          x_t = [wp.tile([128, N], f32, tag=f"x{c}", name=f"x{c}")
                 for c in range(3)]
          for c in range(3):
              for (n0, nn) in NW5:
                  nc.sync.dma_start(x_t[c][:, n0:n0 + nn],
                                    x_d[c * 128:(c + 1) * 128, n0:n0 + nn])
          xq_t = wp.tile([128, 4 * N], fp8, tag="xq", name="xq")
          xqv = xq_t[:].rearrange("p (s n) -> p s n", s=4)
          if it == 0:
              nc.sync.dma_start(xq_t[:], xq_d)
          else:
              nc.sync.dma_start(xqv[:, 3, :], zeros_d)
              for c in range(3):
                  nc.gpsimd.tensor_copy(xqv[:, c, :], x_t[c][:])

          x1_t = [wp.tile([128, N], f32, tag=f"x1_{c}", name=f"x1_{c}")
                  for c in range(3)]
          x1q_t = wp.tile([128, 4 * N], fp8, tag="x1q", name="x1q")
          x1qv = x1q_t[:].rearrange("p (s n) -> p s n", s=4)
          nc.sync.dma_start(x1qv[:, 3, :], zeros_d)

          Gtv = Gt_s[:].rearrange("p (s n) -> p s n", s=4)
          wpvv = wpvT_s[:].rearrange("p (s n) -> p s n", s=4)
          onev = ones_s[:].rearrange("p (s n) -> p s n", s=2)
          f1wv = f1wT_s[:].rearrange("p (s n) -> p s n", s=4)
          f2wv = f2wT_s[:].rearrange("p (s n) -> p s n", s=12)

          # ================= attention =================
          with ExitStack() as attn:
              ap_ = attn.enter_context(tc.tile_pool(name="ap", bufs=1))
              rbp = attn.enter_context(tc.tile_pool(name="rbp", bufs=2))
              tmp_ = attn.enter_context(tc.tile_pool(name="tmp", bufs=2))
              zq_t = ap_.tile([128, 4 * N], fp8, tag="zq", name="zq")
              zqv = zq_t[:].rearrange("p (s n) -> p s n", s=4)
              nc.sync.dma_start(zqv[:, 3, :], zeros_d)
              e_t = [ap_.tile([128, 2 * N], fp8, tag=f"e{q}", name=f"e{q}")
                     for q in range(9)]
              ev = [e_t[q][:].rearrange("p (s n) -> p s n", s=2)
                    for q in range(9)]
              vt_t = [ap_.tile([128, 768], fp8, tag=f"vt{q}", name=f"vt{q}")
                      for q in range(9)]
              vtv = [vt_t[q][:].rearrange("p (s n) -> p s n", s=2)
                     for q in range(9)]

              # z = GG*(A x + u) -> zq   (cast adds per-partition u bias)
              pre = attn.enter_context(ExitStack())
              pzp = pre.enter_context(
                  tc.tile_pool(name="pzp", bufs=2, space="PSUM"))
              for cb in range(3):
                  for (n0, nn) in NW5:
                      pz = pzp.tile([128, 512], f32, tag="pz", name="pz")
                      for pr in range(2):
                          nc.tensor.matmul(
                              pz[:, :nn],
                              Gtv[:, 2 * pr:2 * pr + 2,
                                  cb * 128:(cb + 1) * 128],
                              xqv[:, 2 * pr:2 * pr + 2, n0:n0 + nn],
                              start=(pr == 0), stop=(pr == 1),
                              perf_mode=PM.DoubleRow)
                      nc.scalar.activation(zqv[:, cb, n0:n0 + nn], pz[:, :nn],
                                           AF.Identity,
                                           bias=ub_s[:, cb:cb + 1])

              # v'T[m, c] (transposed, fp8, pair tiles)
              for mb in range(18):
                  pv = pzp.tile([128, 512], f32, tag="pz", name="pz")
                  for pr in range(2):
                      nc.tensor.matmul(
                          pv[:, :384],
                          xqv[:, 2 * pr:2 * pr + 2, mb * 128:(mb + 1) * 128],
                          wpvv[:, 2 * pr:2 * pr + 2, :],
                          start=(pr == 0), stop=(pr == 1),
                          perf_mode=PM.DoubleRow)
                  nc.vector.tensor_copy(
                      vtv[mb // 2][:, mb % 2, :], pv[:, :384])
              pre.close()
              psp = attn.enter_context(
                  tc.tile_pool(name="psp", bufs=2, space="PSUM"))
              plp = attn.enter_context(
                  tc.tile_pool(name="plp", bufs=1, space="PSUM"))
              pup = attn.enter_context(
                  tc.tile_pool(name="pup", bufs=2, space="PSUM"))

              # main softmax-attention loop over query windows
              for (n0, nn) in NW5:
                  pl = plp.tile([128, 512], f32, tag="pl", name="pl")
                  for q in range(9):
                      ps = psp.tile([128, 1024], f32, tag="ps", name="ps")
                      psv = ps[:].rearrange("p (s n) -> p s n", s=2)
                      for half in range(2):
                          mb = 2 * q + half
                          for pr in range(2):
                              nc.tensor.matmul(
                                  psv[:, half, :nn],
                                  xqv[:, 2 * pr:2 * pr + 2,
                                      mb * 128:(mb + 1) * 128],
                                  zqv[:, 2 * pr:2 * pr + 2, n0:n0 + nn],
                                  start=(pr == 0), stop=(pr == 1),
                                  perf_mode=PM.DoubleRow)
                      nc.scalar.activation(ev[q][:, :, n0:n0 + nn],
                                           psv[:, :, :nn], AF.Exp,
                                           scale=exp_scale,
                                           bias=sneg_s[:, 0:1])
                      nc.tensor.matmul(pl[:, :nn], onev,
                                       ev[q][:, :, n0:n0 + nn],
                                       start=(q == 0), stop=(q == 8),
                                       perf_mode=PM.DoubleRow)
                  rbc = rbp.tile([128, 512], bf16, tag="rbc", name="rbc")
                  with nc.allow_low_precision(reason="softmax recip"):
                      nc.vector.reciprocal(rbc[:, :nn], pl[:, :nn])
                  for cb in range(3):
                      pu = pup.tile([128, 512], f32, tag="pu", name="pu")
                      for q in range(9):
                          nc.tensor.matmul(
                              pu[:, :nn],
                              vtv[q][:, :, cb * 128:(cb + 1) * 128],
                              ev[q][:, :, n0:n0 + nn],
                              start=(q == 0), stop=(q == 8),
                              perf_mode=PM.DoubleRow)
                      tm = tmp_.tile([128, 512], f32, tag="tm", name="tm")
                      nc.vector.tensor_tensor(
                          tm[:, :nn], pu[:, :nn], rbc[:, :nn],
                          op=ALU.mult)
                      nc.vector.scalar_tensor_tensor(
                          x1_t[cb][:, n0:n0 + nn], x_t[cb][:, n0:n0 + nn],
                          al1_s[:, cb:cb + 1], tm[:, :nn],
                          op0=ALU.mult, op1=ALU.add)
                      nc.gpsimd.tensor_copy(x1qv[:, cb, n0:n0 + nn],
                                            x1_t[cb][:, n0:n0 + nn])

          # ================= MLP =================
          with ExitStack() as mlp:
              hp = mlp.enter_context(tc.tile_pool(name="hp", bufs=1))
              gp = mlp.enter_context(tc.tile_pool(name="gp", bufs=1))
              otp = mlp.enter_context(tc.tile_pool(name="otp", bufs=4))
              pf1 = mlp.enter_context(
                  tc.tile_pool(name="pf1", bufs=2, space="PSUM"))
              pdw = mlp.enter_context(
                  tc.tile_pool(name="pdw", bufs=1, space="PSUM"))
              pf2 = mlp.enter_context(
                  tc.tile_pool(name="pf2", bufs=2, space="PSUM"))

              h_t = [hp.tile([128, 5120], fp8, tag=f"h{c}", name=f"h{c}")
                     for c in range(12)]
              g_t = [gp.tile([128, 2 * N], fp8, tag=f"g{q}", name=f"g{q}")
                     for q in range(6)]
              gv = [g_t[q][:].rearrange("p (s n) -> p s n", s=2)
                    for q in range(6)]

              for hc in range(12):
                  hv = h_t[hc][:, 0:2500].rearrange("p (y x) -> p y x", y=PAD)
                  nc.gpsimd.memset(hv[:, 0, :], 0.0)
                  nc.gpsimd.memset(hv[:, PAD - 1, :], 0.0)
                  nc.gpsimd.memset(hv[:, :, 0], 0.0)
                  nc.gpsimd.memset(hv[:, :, PAD - 1], 0.0)

              def fc1_chunk(hc):
                  hv = h_t[hc][:, 0:2500].rearrange(
                      "p (y x) -> p y x", y=PAD)
                  for t in range(3):
                      p1 = pf1.tile([128, 1024], f32, tag="p1", name="p1")
                      for w in range(2):
                          nb = t * 768 + w * 384
                          for pr in range(2):
                              nc.tensor.matmul(
                                  p1[:, w * 512:w * 512 + 384],
                                  f1wv[:, 2 * pr:2 * pr + 2,
                                       hc * 128:(hc + 1) * 128],
                                  x1qv[:, 2 * pr:2 * pr + 2, nb:nb + 384],
                                  start=(pr == 0), stop=(pr == 1),
                                  perf_mode=PM.DoubleRow)
                      src = p1[:].rearrange("p (a n) -> p a n", a=2)[
                          :, :, 0:384].rearrange("p a (y x) -> p a y x", y=8)
                      dst = hv[:, 1 + t * 16:1 + t * 16 + 16, 1:49].rearrange(
                          "p (a y) x -> p a y x", a=2)
                      nc.vector.tensor_scalar(
                          dst, src, 1.0 / G1, f1b_s[:, hc:hc + 1],
                          op0=ALU.mult, op1=ALU.add)
                  nc.sync.dma_start(h_t[hc][:, COPY1:COPY1 + 2500],
                                    h_t[hc][:, 0:2500])

              def dw_chunk(hc):
                  for t in range(3):
                      p2 = pdw.tile([128, 1024], f32, tag="p2", name="p2")
                      for w in range(2):
                          y0 = t * 16 + w * 8
                          for p5 in range(5):
                              ta, tb = 2 * p5, min(2 * p5 + 1, 8)
                              offa = (y0 + TAPS[ta][0]) * PAD + TAPS[ta][1]
                              offb = COPY1 + (y0 + TAPS[tb][0]) * PAD \
                                  + TAPS[tb][1]
                              rhs = bass_rust.AP(
                                  h_t[hc][:].tensor, offa,
                                  [[5120, 128], [offb - offa, 2],
                                   [PAD, 8], [1, 48]])
                              nc.tensor.matmul(
                                  p2[:, w * 512:w * 512 + 384],
                                  dwd_s[:, (hc * 5 + p5) * 256:
                                        (hc * 5 + p5) * 256 + 256].rearrange(
                                      "p (s m) -> p s m", s=2),
                                  rhs, start=(p5 == 0), stop=(p5 == 4),
                                  perf_mode=PM.DoubleRow)
                      src = p2[:].rearrange("p (a n) -> p a n", a=2)[
                          :, :, 0:384].rearrange("p a (y x) -> p a y x", y=8)
                      dst = gv[hc // 2][:, hc % 2, t * 768:t * 768 + 768]\
                          .rearrange("p (a y x) -> p a y x", a=2, y=8)
                      nc.scalar.activation(dst, src, AF.Gelu,
                                           bias=dwb_s[:, hc:hc + 1])

              for hc in range(13):
                  if hc < 12:
                      fc1_chunk(hc)
                  if hc >= 1:
                      dw_chunk(hc - 1)

              for (n0, nn) in NT6:
                  for cb in range(3):
                      p3 = pf2.tile([128, 512], f32, tag="p3", name="p3")
                      for q in range(6):
                          nc.tensor.matmul(
                              p3[:, :nn],
                              f2wv[:, 2 * q:2 * q + 2,
                                   cb * 128:(cb + 1) * 128],
                              gv[q][:, :, n0:n0 + nn],
                              start=(q == 0), stop=False,
                              perf_mode=PM.DoubleRow)
                      nc.tensor.matmul(
                          p3[:, :nn],
                          resb_s[:, cb * 128:(cb + 1) * 128],
                          onesb_s[:, :nn], start=False, stop=True)
                      ot = otp.tile([128, 384], f32, tag="ot", name="ot")
                      nc.vector.scalar_tensor_tensor(
                          ot[:, :nn], p3[:, :nn], 1.0 / G2,
                          x1_t[cb][:, n0:n0 + nn],
                          op0=ALU.mult, op1=ALU.add)
                      nc.sync.dma_start(
                          out_d[cb * 128:(cb + 1) * 128, n0:n0 + nn],
                          ot[:, :nn])

    nc.compile()
    return nc


def _f8(a):
    return np.clip(np.asarray(a, np.float32), -240.0, 240.0).astype(F8)


def _fold_inputs(inputs):
    """Host-side folding. Returns (shared weights, per-core dicts)."""
    f = np.float32
    g = {k: np.asarray(v, f) for k, v in inputs.items()}
    s1 = g['bn1_g'] / np.sqrt(g['bn1_v'] + EPS)
    t1 = g['bn1_b'] - g['bn1_m'] * s1
    qw = g['q_w'] * s1[None, :]
    qb = g['q_w'] @ t1 + g['q_b']
    kw = g['k_w'] * s1[None, :]
    A = kw.T @ qw                       # scoresT = x^T A x
    u = kw.T @ qb
    vw = g['v_w'] * s1[None, :]
    vb_eff = g['v_w'] @ t1 + g['v_b']
    ls1, ls2 = g['ls1'], g['ls2']
    Wpv = ls1[:, None] * (g['po_w'] @ vw)
    alpha1 = 1.0 + ls1 * s1
    beta1 = ls1 * (t1 + g['po_b'] + g['po_w'] @ vb_eff)
    s2 = g['bn2_g'] / np.sqrt(g['bn2_v'] + EPS)
    t2 = g['bn2_b'] - g['bn2_m'] * s2
    f1w = g['fc1_w'] * s2[None, :]
    f1b = g['fc1_w'] @ t2 + g['fc1_b'] + f1w @ beta1
    f2w = ls2[:, None] * g['fc2_w']
    resb = beta1 + ls2 * g['fc2_b']
    dww = g['dw_w'].reshape(HID, 9)

    # dw diag pair stationaries: [128, hc, pair, slot, m]
    dwd = np.zeros((128, 12, 5, 2, 128), f)
    for hc in range(12):
        wch = dww[hc * 128:(hc + 1) * 128]  # [128, 9]
        for pr in range(5):
            for i in range(2):
                tap = 2 * pr + i
                if tap <= 8:
                    np.fill_diagonal(dwd[:, hc, pr, i, :], wch[:, tap])

    def chunk_cols(v, k):
        return np.ascontiguousarray(v.reshape(k, 128).T)

    def pad4(m2d):  # [C, X] -> [128, 4, X] with zero slot 3
        Xw = m2d.shape[1]
        out = np.zeros((128, 4, Xw), f)
        out[:, :3, :] = m2d.reshape(3, 128, Xw).transpose(1, 0, 2)
        return out

    w = {
        'Gt': _f8(pad4(GG * (qw.T @ kw)).reshape(128, 4 * 384)),
        'wpvT': _f8(pad4(GV * Wpv.T).reshape(128, 4 * 384)),
        'ones128': np.full((128, 256), 128.0, F8),
        'f1wT': _f8(pad4(G1 * f1w.T).reshape(128, 4 * HID)),
        'f2wT': _f8(np.ascontiguousarray(
            (G2 * f2w.T).reshape(12, 128, 384).transpose(1, 0, 2)
        ).reshape(128, 12 * 384)),
        'dwd': _f8(dwd.reshape(128, 15360)),
        'ub': np.ascontiguousarray(GG * u.reshape(3, 128).T),
        'al1': chunk_cols(alpha1, 3),
        'f1b': chunk_cols(f1b, 12),
        'dwb': chunk_cols(g['dw_b'], 12),
        'resbr': np.ascontiguousarray(
            (G2 * resb)[None, :]).astype(BF16),
        'onesb': np.ones((1, 384), BF16),
        'zeros8': np.zeros((128, N), F8),
    }

    xs = g['x']
    B = xs.shape[0]
    percore = []
    for b in range(B):
        xc = np.ascontiguousarray(xs[b].reshape(C, N))
        z = A @ xc                       # [C, N]
        b_vec = u @ xc                   # [N]
        idx = np.arange(0, N, 18)
        sub = xc.T @ z[:, idx] + b_vec[:, None]
        S = float(sub.max() / SQC + 1.0)
        xq = np.zeros((128, 4, N), F8)
        xq[:, :3, :] = _f8(xc.reshape(3, 128, N).transpose(1, 0, 2))
        percore.append({
            'x': xc,
            'xq': np.ascontiguousarray(xq.reshape(128, 4 * N)),
            'sneg': np.full((128, 1), -S, np.float32),
        })
    return w, percore


def get_program():
    global _PROG
    if _PROG is None:
        _PROG = _build_program()
    return _PROG


def kernel(**inputs):
    from concourse.bass_utils import run_bass_kernel_spmd
    nc = get_program()
    w, percore = _fold_inputs(inputs)
    B = len(percore)
    in_maps = [{**w, **percore[b]} for b in range(B)]
    res = run_bass_kernel_spmd(nc, in_maps, list(range(B)))
    out = np.stack([res.results[b]['out'].reshape(C, H, W) for b in range(B)])
    return out.astype(inputs['x'].dtype if hasattr(inputs['x'], 'dtype')
                      else np.float32)


# revision 8
# speedup vs baseline: 2.5793x; 1.6291x over previous
"""Trainium2 Bass kernel for nn_Block_27384711479862 (metaformer block).

fp8e4 DoubleRow rewrite. Per-core computation (data parallel, B=8 -> 8 cores),
x: [C=384, N=2304]:

  attention (bn1/qkv/proj folded on host):
    zhat = GG*(A @ x + u 1^T)          A = kw^T qw,  u = kw^T qb   (fp8, DR)
    E[m,n] = exp(sum_c x[c,m] zhat[c,n] / (GG sqrt(C)) - S)        (Act)
    l = 128 * sum_m E                  (ones-stationary DR matmul)
    r = 1/l                            (= 1/(GV l) with GV=128)
    v'(c,m) = GV * (ls1*po_w@vw) x     (DR, transposed layout)
    x1 = alpha1*x + (v' E) * r         (2 DVE ops; beta1 deferred)
  mlp (bn2 folded):
    h = fp8(fc1 x1 / G1 + f1b)         (DR; padded [50,50] layout, 2 copies)
    dw = 3x3 depthwise via 5 DR tap-pair diag matmuls (pairs span the
         duplicated h copy so moving-AP windows never overlap)
    g = fp8(gelu(dw + dwb))
    out = (fc2 g + resb)/G2 + x1       (DR + K=1 bf16 bias row)
"""
import numpy as np
import ml_dtypes

C = 384
HID = 1536
H = W = 48
N = H * W              # 2304
PAD = 50
EPS = 1e-5
BF16 = ml_dtypes.bfloat16
F8 = ml_dtypes.float8_e4m3

GG = 8.0      # scores/z scale
GV = 128.0    # Wpv scale (must equal the ones-stationary value)
G1 = 16.0     # fc1 scale
G2 = 256.0    # fc2 scale
SQC = float(np.sqrt(np.float32(C)))

NW5 = [(i * 512, min(512, N - i * 512)) for i in range((N + 511) // 512)]
NT6 = [(i * 384, 384) for i in range(6)]
COPY1 = 2560           # second h copy offset inside the [128, 5120] tile
TAPS = [(t // 3, t % 3) for t in range(9)]

_PROG = None


def _build_program(iters=1):
    import concourse.bacc as bacc
    import concourse.mybir as mybir
    import concourse.tile as tile
    import bass_rust
    from contextlib import ExitStack

    dt = mybir.dt
    AF = mybir.ActivationFunctionType
    ALU = mybir.AluOpType
    PM = mybir.MatmulPerfMode
    f32, bf16, fp8 = dt.float32, dt.bfloat16, dt.float8e4

    nc = bacc.Bacc("TRN2", target_bir_lowering=False, debug=False,
                   enable_asserts=False)

    def din(name, shape, d=f32):
        return nc.dram_tensor(name, list(shape), d, kind="ExternalInput").ap()

    x_d = din("x", (C, N))
    xq_d = din("xq", (128, 4 * N), fp8)
    zeros_d = din("zeros8", (128, N), fp8)
    Gt_d = din("Gt", (128, 4 * 384), fp8)
    wpvT_d = din("wpvT", (128, 4 * 384), fp8)
    ones128_d = din("ones128", (128, 256), fp8)
    f1wT_d = din("f1wT", (128, 4 * HID), fp8)
    f2wT_d = din("f2wT", (128, 12 * 384), fp8)
    dwd_d = din("dwd", (128, 15360), fp8)
    ub_d = din("ub", (128, 3))
    sneg_d = din("sneg", (128, 1))
    al1_d = din("al1", (128, 3))
    f1b_d = din("f1b", (128, 12))
    dwb_d = din("dwb", (128, 12))
    resb_d = din("resbr", (1, 384), bf16)
    onesb_d = din("onesb", (1, 384), bf16)
    out_d = nc.dram_tensor("out", [C, N], f32, kind="ExternalOutput").ap()
    chain = [x_d]
    for i in range(1, iters):
        chain.append(nc.dram_tensor(f"mid{i}", [C, N], f32).ap())
    chain.append(out_d)

    exp_scale = float(1.0 / (GG * SQC))

    with tile.TileContext(nc) as tc:
      with ExitStack() as prog:
        wp = prog.enter_context(tc.tile_pool(name="wp", bufs=1))

        def load(name, shape, d, ap):
            t = wp.tile(list(shape), d, tag=name, name=name)
            nc.sync.dma_start(t[:], ap)
            return t

        Gt_s = load("Gt", (128, 4 * 384), fp8, Gt_d)
        wpvT_s = load("wpvT", (128, 4 * 384), fp8, wpvT_d)
        ones_s = load("ones128", (128, 256), fp8, ones128_d)
        ub_s = load("ub", (128, 3), f32, ub_d)
        sneg_s = load("sneg", (128, 1), f32, sneg_d)
        al1_s = load("al1", (128, 3), f32, al1_d)
        f1wT_s = load("f1wT", (128, 4 * HID), fp8, f1wT_d)
        f2wT_s = load("f2wT", (128, 12 * 384), fp8, f2wT_d)
        dwd_s = load("dwd", (128, 15360), fp8, dwd_d)
        f1b_s = load("f1b", (128, 12), f32, f1b_d)
        dwb_s = load("dwb", (128, 12), f32, dwb_d)
        resb_s = load("resbr", (1, 384), bf16, resb_d)
        onesb_s = load("onesb", (1, 384), bf16, onesb_d)

        Gtv = Gt_s[:].rearrange("p (s n) -> p s n", s=4)
        wpvv = wpvT_s[:].rearrange("p (s n) -> p s n", s=4)
        onev = ones_s[:].rearrange("p (s n) -> p s n", s=2)
        f1wv = f1wT_s[:].rearrange("p (s n) -> p s n", s=4)
        f2wv = f2wT_s[:].rearrange("p (s n) -> p s n", s=12)

        # persistent activations (xq ping-pongs across chain iters)
        xq2 = [wp.tile([128, 4 * N], fp8, tag=f"xq{i}", name=f"xq{i}")
               for i in range(2)]
        nc.gpsimd.dma_start(xq2[0][:], xq_d)
        xqv2 = [t[:].rearrange("p (s n) -> p s n", s=4) for t in xq2]
        nc.sync.dma_start(xqv2[1][:, 3, :], zeros_d)
        x_t = [wp.tile([128, N], f32, tag=f"x{c}", name=f"x{c}")
               for c in range(3)]
        x1_t = [wp.tile([128, N], bf16, tag=f"x1_{c}", name=f"x1_{c}")
                for c in range(3)]
        x1q_t = wp.tile([128, 4 * N], fp8, tag="x1q", name="x1q")
        x1qv = x1q_t[:].rearrange("p (s n) -> p s n", s=4)
        nc.sync.dma_start(x1qv[:, 3, :], zeros_d)

        for it in range(iters):
          x_d, out_d = chain[it], chain[it + 1]
          xqv = xqv2[it % 2]
          xqn = xqv2[(it + 1) % 2]
          for c in range(3):
              nc.gpsimd.dma_start(x_t[c][:],
                                  x_d[c * 128:(c + 1) * 128, :])

          # ================= attention =================
          with ExitStack() as attn:
              ap_ = attn.enter_context(tc.tile_pool(name="ap", bufs=1))
              rbp = attn.enter_context(tc.tile_pool(name="rbp", bufs=2))
              tmp_ = attn.enter_context(tc.tile_pool(name="tmp", bufs=2))
              zq_t = ap_.tile([128, 4 * N], fp8, tag="zq", name="zq")
              zqv = zq_t[:].rearrange("p (s n) -> p s n", s=4)
              nc.sync.dma_start(zqv[:, 3, :], zeros_d)
              e_t = [ap_.tile([128, 2 * N], fp8, tag=f"e{q}", name=f"e{q}")
                     for q in range(9)]
              ev = [e_t[q][:].rearrange("p (s n) -> p s n", s=2)
                    for q in range(9)]
              vt_t = [ap_.tile([128, 768], fp8, tag=f"vt{q}", name=f"vt{q}")
                      for q in range(9)]
              vtv = [vt_t[q][:].rearrange("p (s n) -> p s n", s=2)
                     for q in range(9)]

              pre = attn.enter_context(ExitStack())
              pzp = pre.enter_context(
                  tc.tile_pool(name="pzp", bufs=2, space="PSUM"))
              # v'T[m, c] + z = GG*(A x + u), interleaved so the Act
              # engine gets z-cast work while PE streams v'T blocks
              def vt_block(mb):
                  pv = pzp.tile([128, 512], f32, tag="pz", name="pz")
                  for pr in range(2):
                      nc.tensor.matmul(
                          pv[:, :384],
                          xqv[:, 2 * pr:2 * pr + 2, mb * 128:(mb + 1) * 128],
                          wpvv[:, 2 * pr:2 * pr + 2, :],
                          start=(pr == 0), stop=(pr == 1),
                          perf_mode=PM.DoubleRow)
                  nc.vector.tensor_copy(
                      vtv[mb // 2][:, mb % 2, :], pv[:, :384])

              def z_block(cb, nw):
                  n0, nn = NW5[nw]
                  pz = pzp.tile([128, 512], f32, tag="pz", name="pz")
                  for pr in range(2):
                      nc.tensor.matmul(
                          pz[:, :nn],
                          Gtv[:, 2 * pr:2 * pr + 2, cb * 128:(cb + 1) * 128],
                          xqv[:, 2 * pr:2 * pr + 2, n0:n0 + nn],
                          start=(pr == 0), stop=(pr == 1),
                          perf_mode=PM.DoubleRow)
                  nc.scalar.activation(zqv[:, cb, n0:n0 + nn], pz[:, :nn],
                                       AF.Identity,
                                       bias=ub_s[:, cb:cb + 1])

              for k in range(18):
                  if k < 15:
                      z_block(k % 3, k // 3)
                  vt_block(k)
              pre.close()
              psp = attn.enter_context(
                  tc.tile_pool(name="psp", bufs=2, space="PSUM"))
              plp = attn.enter_context(
                  tc.tile_pool(name="plp", bufs=1, space="PSUM"))
              pup = attn.enter_context(
                  tc.tile_pool(name="pup", bufs=2, space="PSUM"))

              # main softmax-attention loop over query windows
              for (n0, nn) in NW5:
                  pl = plp.tile([128, 512], f32, tag="pl", name="pl")
                  for q in range(9):
                      ps = psp.tile([128, 1024], f32, tag="ps", name="ps")
                      psv = ps[:].rearrange("p (s n) -> p s n", s=2)
                      for half in range(2):
                          mb = 2 * q + half
                          for pr in range(2):
                              nc.tensor.matmul(
                                  psv[:, half, :nn],
                                  xqv[:, 2 * pr:2 * pr + 2,
                                      mb * 128:(mb + 1) * 128],
                                  zqv[:, 2 * pr:2 * pr + 2, n0:n0 + nn],
                                  start=(pr == 0), stop=(pr == 1),
                                  perf_mode=PM.DoubleRow)
                      nc.scalar.activation(ev[q][:, :, n0:n0 + nn],
                                           psv[:, :, :nn], AF.Exp,
                                           scale=exp_scale,
                                           bias=sneg_s[:, 0:1])
                      nc.tensor.matmul(pl[:, :nn], onev,
                                       ev[q][:, :, n0:n0 + nn],
                                       start=(q == 0), stop=(q == 8),
                                       perf_mode=PM.DoubleRow)
                  rbc = rbp.tile([128, 512], bf16, tag="rbc", name="rbc")
                  with nc.allow_low_precision(reason="softmax recip"):
                      nc.vector.reciprocal(rbc[:, :nn], pl[:, :nn])
                  for cb in range(3):
                      pu = pup.tile([128, 512], f32, tag="pu", name="pu")
                      for q in range(9):
                          nc.tensor.matmul(
                              pu[:, :nn],
                              vtv[q][:, :, cb * 128:(cb + 1) * 128],
                              ev[q][:, :, n0:n0 + nn],
                              start=(q == 0), stop=(q == 8),
                              perf_mode=PM.DoubleRow)
                      tm = tmp_.tile([128, 512], f32, tag="tm", name="tm")
                      nc.vector.tensor_tensor(
                          tm[:, :nn], pu[:, :nn], rbc[:, :nn],
                          op=ALU.mult)
                      with nc.allow_low_precision(reason="x1 bf16"):
                          nc.vector.scalar_tensor_tensor(
                              x1_t[cb][:, n0:n0 + nn], x_t[cb][:, n0:n0 + nn],
                              al1_s[:, cb:cb + 1], tm[:, :nn],
                              op0=ALU.mult, op1=ALU.add)
                      nc.gpsimd.tensor_copy(x1qv[:, cb, n0:n0 + nn],
                                            x1_t[cb][:, n0:n0 + nn])

          # ================= MLP =================
          with ExitStack() as mlp:
              hp = mlp.enter_context(tc.tile_pool(name="hp", bufs=1))
              gp = mlp.enter_context(tc.tile_pool(name="gp", bufs=1))
              otp = mlp.enter_context(tc.tile_pool(name="otp", bufs=4))

              h_t = [hp.tile([128, 5120], fp8, tag=f"h{c}", name=f"h{c}")
                     for c in range(12)]
              g_t = [gp.tile([128, 2 * N], fp8, tag=f"g{q}", name=f"g{q}")
                     for q in range(6)]
              gv = [g_t[q][:].rearrange("p (s n) -> p s n", s=2)
                    for q in range(6)]

              for hc in range(12):
                  hv = h_t[hc][:, 0:2500].rearrange("p (y x) -> p y x", y=PAD)
                  nc.gpsimd.memset(hv[:, 0, :], 0.0)
                  nc.gpsimd.memset(hv[:, PAD - 1, :], 0.0)
                  nc.gpsimd.memset(hv[:, :, 0], 0.0)
                  nc.gpsimd.memset(hv[:, :, PAD - 1], 0.0)

              fdw = mlp.enter_context(ExitStack())
              pf1 = fdw.enter_context(
                  tc.tile_pool(name="pf1", bufs=2, space="PSUM"))
              pdw = fdw.enter_context(
                  tc.tile_pool(name="pdw", bufs=2, space="PSUM"))

              def fc1_chunk(hc):
                  hv = h_t[hc][:, 0:2500].rearrange(
                      "p (y x) -> p y x", y=PAD)
                  for t in range(3):
                      p1 = pf1.tile([128, 1024], f32, tag="p1", name="p1")
                      for w in range(2):
                          nb = t * 768 + w * 384
                          for pr in range(2):
                              nc.tensor.matmul(
                                  p1[:, w * 512:w * 512 + 384],
                                  f1wv[:, 2 * pr:2 * pr + 2,
                                       hc * 128:(hc + 1) * 128],
                                  x1qv[:, 2 * pr:2 * pr + 2, nb:nb + 384],
                                  start=(pr == 0), stop=(pr == 1),
                                  perf_mode=PM.DoubleRow)
                      src = p1[:].rearrange("p (a n) -> p a n", a=2)[
                          :, :, 0:384].rearrange("p a (y x) -> p a y x", y=8)
                      dst = hv[:, 1 + t * 16:1 + t * 16 + 16, 1:49].rearrange(
                          "p (a y) x -> p a y x", a=2)
                      nc.vector.tensor_scalar(
                          dst, src, 1.0 / G1, f1b_s[:, hc:hc + 1],
                          op0=ALU.mult, op1=ALU.add)
                  nc.gpsimd.dma_start(h_t[hc][:, COPY1:COPY1 + 2500],
                                      h_t[hc][:, 0:2500])

              def dw_chunk(hc):
                  for t in range(3):
                      p2 = pdw.tile([128, 1024], f32, tag="p2", name="p2")
                      for w in range(2):
                          y0 = t * 16 + w * 8
                          for p5 in range(5):
                              ta, tb = 2 * p5, min(2 * p5 + 1, 8)
                              offa = (y0 + TAPS[ta][0]) * PAD + TAPS[ta][1]
                              offb = COPY1 + (y0 + TAPS[tb][0]) * PAD \
                                  + TAPS[tb][1]
                              rhs = bass_rust.AP(
                                  h_t[hc][:].tensor, offa,
                                  [[5120, 128], [offb - offa, 2],
                                   [PAD, 8], [1, 48]])
                              nc.tensor.matmul(
                                  p2[:, w * 512:w * 512 + 384],
                                  dwd_s[:, (hc * 5 + p5) * 256:
                                        (hc * 5 + p5) * 256 + 256].rearrange(
                                      "p (s m) -> p s m", s=2),
                                  rhs, start=(p5 == 0), stop=(p5 == 4),
                                  perf_mode=PM.DoubleRow)
                      src = p2[:].rearrange("p (a n) -> p a n", a=2)[
                          :, :, 0:384].rearrange("p a (y x) -> p a y x", y=8)
                      dst = gv[hc // 2][:, hc % 2, t * 768:t * 768 + 768]\
                          .rearrange("p (a y x) -> p a y x", a=2, y=8)
                      nc.scalar.activation(dst, src, AF.Gelu,
                                           bias=dwb_s[:, hc:hc + 1])

              for hc in range(13):
                  if hc < 12:
                      fc1_chunk(hc)
                  if hc >= 1:
                      dw_chunk(hc - 1)
              fdw.close()

              pf2 = mlp.enter_context(
                  tc.tile_pool(name="pf2", bufs=4, space="PSUM"))
              for (n0, nn) in NT6:
                  for cb in range(3):
                      p3 = pf2.tile([128, 512], f32, tag="p3", name="p3")
                      for q in range(6):
                          nc.tensor.matmul(
                              p3[:, :nn],
                              f2wv[:, 2 * q:2 * q + 2,
                                   cb * 128:(cb + 1) * 128],
                              gv[q][:, :, n0:n0 + nn],
                              start=(q == 0), stop=False,
                              perf_mode=PM.DoubleRow)
                      nc.tensor.matmul(
                          p3[:, :nn],
                          resb_s[:, cb * 128:(cb + 1) * 128],
                          onesb_s[:, :nn], start=False, stop=True)
                      ot = otp.tile([128, 384], f32, tag="ot", name="ot")
                      nc.vector.scalar_tensor_tensor(
                          ot[:, :nn], p3[:, :nn], 1.0 / G2,
                          x1_t[cb][:, n0:n0 + nn],
                          op0=ALU.mult, op1=ALU.add)
                      nc.sync.dma_start(
                          out_d[cb * 128:(cb + 1) * 128, n0:n0 + nn],
                          ot[:, :nn])
                      if it + 1 < iters:
                          nc.gpsimd.tensor_copy(xqn[:, cb, n0:n0 + nn],
                                                ot[:, :nn])

    nc.compile()
    return nc


def _f8(a):
    return np.clip(np.asarray(a, np.float32), -240.0, 240.0).astype(F8)


def _fold_inputs(inputs):
    """Host-side folding. Returns (shared weights, per-core dicts)."""
    f = np.float32
    g = {k: np.asarray(v, f) for k, v in inputs.items()}
    s1 = g['bn1_g'] / np.sqrt(g['bn1_v'] + EPS)
    t1 = g['bn1_b'] - g['bn1_m'] * s1
    qw = g['q_w'] * s1[None, :]
    qb = g['q_w'] @ t1 + g['q_b']
    kw = g['k_w'] * s1[None, :]
    A = kw.T @ qw                       # scoresT = x^T A x
    u = kw.T @ qb
    vw = g['v_w'] * s1[None, :]
    vb_eff = g['v_w'] @ t1 + g['v_b']
    ls1, ls2 = g['ls1'], g['ls2']
    Wpv = ls1[:, None] * (g['po_w'] @ vw)
    alpha1 = 1.0 + ls1 * s1
    beta1 = ls1 * (t1 + g['po_b'] + g['po_w'] @ vb_eff)
    s2 = g['bn2_g'] / np.sqrt(g['bn2_v'] + EPS)
    t2 = g['bn2_b'] - g['bn2_m'] * s2
    f1w = g['fc1_w'] * s2[None, :]
    f1b = g['fc1_w'] @ t2 + g['fc1_b'] + f1w @ beta1
    f2w = ls2[:, None] * g['fc2_w']
    resb = beta1 + ls2 * g['fc2_b']
    dww = g['dw_w'].reshape(HID, 9)

    # dw diag pair stationaries: [128, hc, pair, slot, m]
    dwd = np.zeros((128, 12, 5, 2, 128), f)
    for hc in range(12):
        wch = dww[hc * 128:(hc + 1) * 128]  # [128, 9]
        for pr in range(5):
            for i in range(2):
                tap = 2 * pr + i
                if tap <= 8:
                    np.fill_diagonal(dwd[:, hc, pr, i, :], wch[:, tap])

    def chunk_cols(v, k):
        return np.ascontiguousarray(v.reshape(k, 128).T)

    def pad4(m2d):  # [C, X] -> [128, 4, X] with zero slot 3
        Xw = m2d.shape[1]
        out = np.zeros((128, 4, Xw), f)
        out[:, :3, :] = m2d.reshape(3, 128, Xw).transpose(1, 0, 2)
        return out

    w = {
        'Gt': _f8(pad4(GG * (qw.T @ kw)).reshape(128, 4 * 384)),
        'wpvT': _f8(pad4(GV * Wpv.T).reshape(128, 4 * 384)),
        'ones128': np.full((128, 256), 128.0, F8),
        'f1wT': _f8(pad4(G1 * f1w.T).reshape(128, 4 * HID)),
        'f2wT': _f8(np.ascontiguousarray(
            (G2 * f2w.T).reshape(12, 128, 384).transpose(1, 0, 2)
        ).reshape(128, 12 * 384)),
        'dwd': _f8(dwd.reshape(128, 15360)),
        'ub': np.ascontiguousarray(GG * u.reshape(3, 128).T),
        'al1': chunk_cols(alpha1, 3),
        'f1b': chunk_cols(f1b, 12),
        'dwb': chunk_cols(g['dw_b'], 12),
        'resbr': np.ascontiguousarray(
            (G2 * resb)[None, :]).astype(BF16),
        'onesb': np.ones((1, 384), BF16),
        'zeros8': np.zeros((128, N), F8),
    }

    xs = g['x']
    B = xs.shape[0]
    percore = []
    for b in range(B):
        xc = np.ascontiguousarray(xs[b].reshape(C, N))
        z = A @ xc                       # [C, N]
        b_vec = u @ xc                   # [N]
        idx = np.arange(0, N, 18)
        sub = xc.T @ z[:, idx] + b_vec[:, None]
        S = float(sub.max() / SQC + 1.0)
        xq = np.zeros((128, 4, N), F8)
        xq[:, :3, :] = _f8(xc.reshape(3, 128, N).transpose(1, 0, 2))
        percore.append({
            'x': xc,
            'xq': np.ascontiguousarray(xq.reshape(128, 4 * N)),
            'sneg': np.full((128, 1), -S, np.float32),
        })
    return w, percore


def get_program():
    global _PROG
    if _PROG is None:
        _PROG = _build_program()
    return _PROG


def kernel(**inputs):
    from concourse.bass_utils import run_bass_kernel_spmd
    nc = get_program()
    w, percore = _fold_inputs(inputs)
    B = len(percore)
    in_maps = [{**w, **percore[b]} for b in range(B)]
    res = run_bass_kernel_spmd(nc, in_maps, list(range(B)))
    out = np.stack([res.results[b]['out'].reshape(C, H, W) for b in range(B)])
    return out.astype(inputs['x'].dtype if hasattr(inputs['x'], 'dtype')
                      else np.float32)
